# revision 5
# baseline (speedup 1.0000x reference)
"""Multi-head attention kernel for Trainium2, sharded over 8 NeuronCores.

Problem: B=4, S=2048, D=256, H=8 dense transformer attention block
(per-head K/V/Q Linear projections + dot-product attention + output Linear).

Sharding: core = (batch b, head-group g); core 2*b+g handles batch b and
heads [4g, 4g+4). Each core computes its heads' contribution to the final
output Linear (Wo rows h::H belong to head h); the host sums the two
partial outputs per batch and adds the (host-folded) bias.

Algebraic folds (host-side, exact up to rounding):
  - scores = k M q^T + ku[m] + (per-query terms that cancel in softmax),
    with M = 64 * Wk (Wq/16)^T and ku = k (Wk bq/16). The x64 keeps the
    fp8 t-projection in e4m3's normal range; exp() divides it back out
    via its scale operand. No Q projection, no K/Q bias adds on chip.
  - AV+output: w^T (v Wv + bv) Wo_h = w^T (v W2) + bv Wo_h with
    W2 = 16 * Wv Wo_h; bo' = bo + sum_h bv[h] Wo_h is added on host. The
    x16 is cancelled by summing the softmax denominator with a 16-valued
    ones matmul.
  - k/v/q are transposed to [D, S] on host; q is quantized to fp8 e4m3
    and k/v to bf16 on host.

On-chip per core: projections run in bf16 (full PE rate); scores, AV and
the softmax denominator run as fp8 e4m3 DoubleRow matmuls (2 k-tiles per
instruction, 0.5 cycles/column = 4x the bf16 rate). exp() runs on the
Act engine only (no table swaps), one instruction per (key-tile, query
half: [128, 1024] across two PSUM banks), with the folded K-side bias ku
as its per-partition bias operand. Scores are emitted key-tile-major so
one exp covers two query blocks with a single bias column. Softmax
denominators come from a 16-valued fp8 DoubleRow ones-matmul on the PE.
The Act engine is the critical path; score/AV/projection matmuls are
interleaved in emission order to fill the PE's Act-gated gaps.
"""

import numpy as np
import ml_dtypes
from contextlib import ExitStack

import concourse.bacc as bacc
import concourse.bass as bass
import concourse.tile as tile
from concourse import mybir
from concourse.bass_utils import run_bass_kernel_spmd

B, S, D, H = 4, 2048, 256, 8
P = 128
DC = D // P            # 2 contraction/e-tile chunks
HPC = H // 2           # 4 heads per core
QB = 512               # query-block width
NQB = S // QB          # 4 query blocks
MT = S // P            # 16 key tiles
F32 = mybir.dt.float32
BF16 = mybir.dt.bfloat16
E4 = mybir.dt.float8e4
EXP = mybir.ActivationFunctionType.Exp
DR = mybir.MatmulPerfMode.DoubleRow


def build_program(repeat=1, nwarm=24):
    nc = bacc.Bacc(None, target_bir_lowering=False)

    ktd = nc.dram_tensor("kt", [D, S], BF16, kind="ExternalInput")
    vtd = nc.dram_tensor("vt", [D, S], BF16, kind="ExternalInput")
    qtd = nc.dram_tensor("qt", [D, S], E4, kind="ExternalInput")
    wmd = nc.dram_tensor("wm", [HPC, D, D], BF16, kind="ExternalInput")
    w2d = nc.dram_tensor("w2", [HPC, D, D], BF16, kind="ExternalInput")
    kud = nc.dram_tensor("ku", [HPC, P, MT], F32, kind="ExternalInput")
    outd = nc.dram_tensor("out", [D, S], F32, kind="ExternalOutput")

    with ExitStack() as ctx:
        tc = ctx.enter_context(tile.TileContext(nc))
        const = ctx.enter_context(tc.tile_pool(name="const", bufs=1))
        wpool = ctx.enter_context(tc.tile_pool(name="w", bufs=2))
        # V2 needs 3 bufs: head h's V2 is read by av_nb during head h+1's
        # slots, while the projection for head h+2 writes a third buffer.
        kqv = ctx.enter_context(tc.tile_pool(name="kqv", bufs=3))
        epool = ctx.enter_context(tc.tile_pool(name="exp", bufs=2))
        rcpool = ctx.enter_context(tc.tile_pool(name="recip", bufs=2))
        scpool = ctx.enter_context(tc.tile_pool(name="scratch", bufs=2))
        psE = ctx.enter_context(
            tc.tile_pool(name="psE", bufs=2, space=bass.MemorySpace.PSUM))
        psR = ctx.enter_context(
            tc.tile_pool(name="psR", bufs=2, space=bass.MemorySpace.PSUM))
        psM = ctx.enter_context(
            tc.tile_pool(name="psM", bufs=2, space=bass.MemorySpace.PSUM))

        ones_w = const.tile([P, P], BF16)
        nc.vector.memset(ones_w[:], 1.0)
        ones16 = const.tile([P, 2, P], E4)
        nc.vector.memset(ones16[:], 16.0)

        for _rep in range(repeat):
            _build_iteration(nc, const, wpool, kqv, epool, rcpool, scpool,
                             psE, psR, psM, ones_w, ones16,
                             ktd, vtd, qtd, wmd, w2d, kud, outd, nwarm)

    nc.compile()
    return nc


def _build_iteration(nc, const, wpool, kqv, epool, rcpool, scpool,
                     psE, psR, psM, ones_w, ones16,
                     ktd, vtd, qtd, wmd, w2d, kud, outd, nwarm=24):
    # Warm the PE through the cold p-state window during the input-DMA wait.
    ps_warm = psM.tile([P, QB], F32, tag="psM")
    for wi in range(nwarm):
        nc.tensor.matmul(ps_warm[:, :P], ones_w[:], ones_w[:],
                         start=(wi == 0), stop=(wi == nwarm - 1))

    def load_weights(h):
        wm_sb = wpool.tile([P, DC, D], BF16, tag="wm")
        w2_sb = wpool.tile([P, DC, D], BF16, tag="w2")
        ku_sb = wpool.tile([P, MT], F32, tag="ku")
        for dc in range(DC):
            nc.sync.dma_start(wm_sb[:, dc, :], wmd[h, dc * P:(dc + 1) * P, :])
            nc.sync.dma_start(w2_sb[:, dc, :], w2d[h, dc * P:(dc + 1) * P, :])
        nc.gpsimd.dma_start(ku_sb[:], kud[h])
        return wm_sb, w2_sb, ku_sb

    w_cur = load_weights(0)

    kT = const.tile([P, DC, S], BF16)
    vT = const.tile([P, DC, S], BF16)
    qT = const.tile([P, DC, S], E4)
    # kT chunked mb-major (t-proj group order), qT by query halves.
    for mb in range(NQB):
        for dc in range(DC):
            nc.sync.dma_start(kT[:, dc, mb * QB:(mb + 1) * QB],
                              ktd[dc * P:(dc + 1) * P, mb * QB:(mb + 1) * QB])
    for nbp in range(2):
        for dc in range(DC):
            sl = slice(nbp * 2 * QB, (nbp + 1) * 2 * QB)
            nc.gpsimd.dma_start(qT[:, dc, sl], qtd[dc * P:(dc + 1) * P, sl])
    HS = S // 2
    for half in range(2):
        for dc in range(DC):
            sl = slice(half * HS, (half + 1) * HS)
            nc.scalar.dma_start(vT[:, dc, sl], vtd[dc * P:(dc + 1) * P, sl])

    # out_acc[p, et, n] accumulates out^T[f = et*128+p, n] over heads
    out_acc = const.tile([P, DC, S], F32)

    def proj_group(g, wm_sb, w2_sb, tT_h, V2_h):
        """One projection PSUM group of head h: g<8 -> t-proj (mb=g//2,
        et=g%2); g>=8 -> V2 (mp=g-8, two m-tiles per bank)."""
        ps = psM.tile([P, QB], F32, tag="psM")
        if g < 8:
            mb, et = g // 2, g % 2
            for dc in range(DC):
                nc.tensor.matmul(
                    ps[:], wm_sb[:, dc, et * P:(et + 1) * P],
                    kT[:, dc, mb * QB:(mb + 1) * QB],
                    start=(dc == 0), stop=(dc == DC - 1))
            nc.vector.tensor_copy(tT_h[:, et, mb * QB:(mb + 1) * QB], ps[:])
        else:
            mp = g - 8
            for half in range(2):
                mt = 2 * mp + half
                for dc in range(DC):
                    nc.tensor.matmul(
                        ps[:, half * D:(half + 1) * D],
                        vT[:, dc, mt * P:(mt + 1) * P],
                        w2_sb[:, dc, :],
                        start=(dc == 0), stop=(dc == DC - 1))
            nc.vector.tensor_copy(V2_h[:, 2 * mp:2 * mp + 2, :], ps[:])

    def scores_slot(nbp, mt, tT_h, ku_sb, expT):
        """Two DR score matmuls (query blocks 2*nbp, 2*nbp+1) for key tile
        mt into one 2-bank PSUM tile, then a single exp over both."""
        ps = psE.tile([P, 2 * QB], F32, tag="psE")
        for half in range(2):
            nb = 2 * nbp + half
            nc.tensor.matmul(
                ps[:, half * QB:(half + 1) * QB],
                tT_h[:, :, mt * P:(mt + 1) * P],
                qT[:, :, nb * QB:(nb + 1) * QB],
                start=True, stop=True, perf_mode=DR)
        nc.scalar.activation(
            expT[:, mt, nbp * 2 * QB:(nbp + 1) * 2 * QB], ps[:],
            EXP, bias=ku_sb[:, mt:mt + 1], scale=1.0 / 64.0)

    def av_nb(h, nb, V2_h, expT):
        """Denominator + AV for query block nb of head h, then normalize
        into out_acc (and DMA out for the last head)."""
        esl = slice(nb * QB, (nb + 1) * QB)
        psS = psM.tile([P, QB], F32, tag="psM")
        for j in range(MT // 2):
            nc.tensor.matmul(psS[:], ones16[:],
                             expT[:, 2 * j:2 * j + 2, esl],
                             start=(j == 0), stop=(j == MT // 2 - 1),
                             perf_mode=DR)
        recip = rcpool.tile([P, QB], F32, tag="recip")
        nc.vector.reciprocal(recip[:], psS[:])
        pair = []
        for et in range(DC):
            ps = psR.tile([P, QB], F32, tag="psR")
            for j in range(MT // 2):
                nc.tensor.matmul(
                    ps[:], V2_h[:, 2 * j:2 * j + 2, et * P:(et + 1) * P],
                    expT[:, 2 * j:2 * j + 2, esl],
                    start=(j == 0), stop=(j == MT // 2 - 1), perf_mode=DR)
            pair.append(ps)
        for et in range(DC):
            osl = out_acc[:, et, esl]
            if h == 0:
                nc.vector.tensor_mul(osl, pair[et][:], recip[:])
            else:
                sc = scpool.tile([P, QB], F32, tag="sc")
                nc.vector.tensor_mul(sc[:], pair[et][:], recip[:])
                nc.gpsimd.tensor_add(osl, osl, sc[:])
            if h == HPC - 1:
                eng = nc.sync if et == 0 else nc.gpsimd
                eng.dma_start(outd[et * P:(et + 1) * P, nb * QB:(nb + 1) * QB],
                              osl)

    tT_cur = kqv.tile([P, DC, S], E4, tag="tT")
    V2_cur = kqv.tile([P, MT, D], E4, tag="V2")
    exp_prev = V2_prev = None
    for h in range(HPC):
        wm_sb, w2_sb, ku_sb = w_cur
        if h == 0:
            # First head: projections run up front, overlapping input DMA.
            for g in range(16):
                proj_group(g, wm_sb, w2_sb, tT_cur, V2_cur)
        if h + 1 < HPC:
            w_next = load_weights(h + 1)
            tT_nxt = kqv.tile([P, DC, S], E4, tag="tT")
            V2_nxt = kqv.tile([P, MT, D], E4, tag="V2")

        expT = epool.tile([P, MT, S], E4, tag="exp")

        # Slot schedule: AV of the previous head and projections of the
        # next head fill the PE's Act-gated gaps between score groups.
        av_slots = {1: (h - 1, 0), 5: (h - 1, 1), 9: (h - 1, 2),
                    13: (h - 1, 3)}
        if h == HPC - 1:
            av_slots.update({18: (h, 0), 23: (h, 1)})
        slot = 0
        for nbp in range(2):
            for mt in range(MT):
                scores_slot(nbp, mt, tT_cur, ku_sb, expT)
                if h + 1 < HPC and slot < 16:
                    proj_group(slot, w_next[0], w_next[1], tT_nxt, V2_nxt)
                if slot in av_slots:
                    ah, anb = av_slots[slot]
                    if ah >= 0:
                        a_e, a_v = ((expT, V2_cur) if ah == h
                                    else (exp_prev, V2_prev))
                        av_nb(ah, anb, a_v, a_e)
                slot += 1

        exp_prev, V2_prev = expT, V2_cur
        if h + 1 < HPC:
            tT_cur, V2_cur = tT_nxt, V2_nxt
            w_cur = w_next

    for nb in (2, 3):
        av_nb(HPC - 1, nb, V2_prev, exp_prev)


_progs = {}


def _get_prog(repeat=1):
    if repeat not in _progs:
        _progs[repeat] = build_program(repeat)
    return _progs[repeat]


def _prepare_in_maps(k, v, q, Wk, bk, Wv, bv, Wq, bq, Wo, bo):
    scale = np.float32(1.0 / 16.0)  # 1/sqrt(D), exact power of two
    in_maps = []
    for core in range(2 * B):
        b, g = core // 2, core % 2
        hs = list(range(g * HPC, (g + 1) * HPC))
        wm = np.stack([
            (Wk[h].astype(np.float64)
             @ (Wq[h].astype(np.float64) * scale).T * 64.0
             ).astype(ml_dtypes.bfloat16)
            for h in hs])
        w2 = np.stack([
            (Wv[h].astype(np.float64)
             @ Wo[h::H].astype(np.float64) * 16.0).astype(ml_dtypes.bfloat16)
            for h in hs])
        ku = np.stack([
            (k[b].astype(np.float64)
             @ (Wk[h].astype(np.float64) @ (bq[h].astype(np.float64) * scale))
             ).astype(np.float32).reshape(MT, P).T
            for h in hs])
        in_maps.append({
            "kt": np.ascontiguousarray(k[b].T).astype(ml_dtypes.bfloat16),
            "vt": np.ascontiguousarray(v[b].T).astype(ml_dtypes.bfloat16),
            "qt": np.ascontiguousarray(q[b].T).astype(ml_dtypes.float8_e4m3),
            "wm": np.ascontiguousarray(wm),
            "w2": np.ascontiguousarray(w2),
            "ku": np.ascontiguousarray(ku),
        })
    return in_maps


def _bo_prime(bv, Wo, bo):
    acc = bo.astype(np.float64).copy()
    for h in range(H):
        acc += bv[h].astype(np.float64) @ Wo[h::H].astype(np.float64)
    return acc.astype(np.float32)


def _run_spmd(in_maps, repeat=1, **kwargs):
    nc = _get_prog(repeat)
    return run_bass_kernel_spmd(nc, in_maps, core_ids=list(range(2 * B)),
                                **kwargs)


def kernel(k, v, q, Wk, bk, Wv, bv, Wq, bq, Wo, bo):
    arrs = [np.asarray(x, dtype=np.float32)
            for x in (k, v, q, Wk, bk, Wv, bv, Wq, bq, Wo, bo)]
    k, v, q, Wk, bk, Wv, bv, Wq, bq, Wo, bo = arrs
    in_maps = _prepare_in_maps(k, v, q, Wk, bk, Wv, bv, Wq, bq, Wo, bo)
    rr = _run_spmd(in_maps)
    bop = _bo_prime(bv, Wo, bo)
    out = np.empty((B, S, D), np.float32)
    for b in range(B):
        out[b] = (rr.results[2 * b]["out"].T + rr.results[2 * b + 1]["out"].T
                  + bop)
    return out
